# revision 23
# baseline (speedup 1.0000x reference)
"""Trainium2 Bass kernel for the LGP-instruction module (read -> op bank -> write).

Data-parallel over batch: core b computes x[b] (2048, 4096).
Precision plan (gate 2e-2, measured ~3.9e-3): x and read_w in fp8-e4m3
(read_w scaled x256, dequant folded into op weights), everything else bf16.
Read phase uses DoubleRow fp8 matmuls (K=256 per instruction).

Pipeline per core, 4 T-chunks of 512:
  x loads in two T-half blocks (4 sub-DMAs each); chunks 0/1 accumulate as
  block 0 streams in, chunk 0's op-bank/write/store phase starts right after,
  with block 1's read matmuls interleaved between chunk-0 write bursts as
  PE filler while drains catch up.
  op bank: ACT {relu,square,abs,gelu,tanh,tanh(=sigmoid)} + DVE {ident,neg},
  mixture summed by a short tree (DVE + GpSimd).
  write: acc.T @ wwT (bf16), drains alternate DVE/ACT, stores alternate
  SWDGE (gpsimd) / HWDGE (sync).
"""
import sys
import numpy as np

if '/opt/trn_rl_repo' not in sys.path:
    sys.path.insert(0, '/opt/trn_rl_repo')

B, T, V, C, NOPS = 8, 2048, 4096, 128, 8
NCORES = 8
NV = V // 128     # 32 v-tiles
NP = NV // 2      # 16 v-tile pairs (DoubleRow)
TC = 512          # T-chunk
NTC = T // TC     # 4 T-chunks
RW_SCALE = 256.0

_CACHE = {}
LAST_RESULT = None


def _build(pre, post):
    from concourse import bass, bacc, tile, mybir
    f32, bf16, f8 = mybir.dt.float32, mybir.dt.bfloat16, mybir.dt.float8e4
    AF = mybir.ActivationFunctionType
    DR = mybir.MatmulPerfMode.DoubleRow
    ts = bass.ts
    alu = mybir.AluOpType

    nc = bacc.Bacc("TRN2", target_bir_lowering=False, debug=False,
                   num_devices=NCORES)
    xT = nc.dram_tensor("xT", [V, T], f8, kind="ExternalInput")
    rw = nc.dram_tensor("rw", [128, NV * C], f8, kind="ExternalInput")
    wwT = nc.dram_tensor("wwT", [C, V], bf16, kind="ExternalInput")
    opw = nc.dram_tensor("opw", [C, NOPS * C], bf16, kind="ExternalInput")
    opb = nc.dram_tensor("opb", [C, NOPS], f32, kind="ExternalInput")
    out = nc.dram_tensor("out", [T, V], bf16, kind="ExternalOutput")

    xTr = xT.ap().rearrange("(vb p) t -> p vb t", p=128)

    with tile.TileContext(nc) as tc:
        with tc.tile_pool(name="const", bufs=1) as constp, \
             tc.tile_pool(name="xt", bufs=1) as xtp, \
             tc.tile_pool(name="vals_ps", bufs=1, space="PSUM") as vpsp, \
             tc.tile_pool(name="vals_sb", bufs=1) as vsbp, \
             tc.tile_pool(name="hw_ps", bufs=4, space="PSUM") as hwpsp, \
             tc.tile_pool(name="t_sb", bufs=10) as tp, \
             tc.tile_pool(name="s_sb", bufs=8) as sp, \
             tc.tile_pool(name="acc", bufs=2) as accp, \
             tc.tile_pool(name="out_sb", bufs=3) as osbp:

            # x-path consts first on the sync queue; the rest on the scalar
            # engine's queue so they never delay x blocks. rw loads as one
            # contiguous 4KB line per partition; matmuls use a 3D view.
            rw_flat = constp.tile([128, NV * C], f8)
            nc.sync.dma_start(rw_flat[:], rw.ap())
            rw_t = rw_flat.rearrange("p (vt c) -> p vt c", c=C)
            wwT_t = constp.tile([C, V], bf16)
            nc.scalar.dma_start(wwT_t[:], wwT.ap())
            opw_t = constp.tile([C, NOPS, C], bf16)
            nc.scalar.dma_start(opw_t[:], opw.ap().rearrange("p (i c) -> p i c", c=C))
            opb_t = constp.tile([C, NOPS], f32)
            nc.scalar.dma_start(opb_t[:], opb.ap())

            # x resident in fp8; two T-half blocks of 4 sub-DMAs each
            xfull = xtp.tile([128, NV, T], f8)
            for bt in range(2):
                for vb in range(4):
                    nc.sync.dma_start(
                        xfull[:, ts(vb, 8), ts(bt, 1024)],
                        xTr[:, ts(vb, 8), ts(bt, 1024)])

            values = [vpsp.tile([128, TC], f32, name=f"values{i}")
                      for i in range(NTC)]

            def read_mms(bt):
                # 32 DoubleRow matmuls covering chunks (2*bt, 2*bt+1)
                g = []
                for vb in range(4):
                    for j in range(4):
                        k = vb * 4 + j
                        for lc in range(2):
                            c = bt * 2 + lc
                            g.append((k, c))
                return g

            def emit_read(k, c):
                nc.tensor.matmul(
                    values[c][:], rw_t[:, 2 * k:2 * k + 2, :],
                    xfull[:, 2 * k:2 * k + 2, ts(c, TC)],
                    start=(k == 0), stop=(k == NP - 1), perf_mode=DR)

            for k, c in read_mms(0):
                emit_read(k, c)

            vals = {}

            def emit_vals_copy(c):
                v = vsbp.tile([128, TC], bf16, name=f"vals{c}")
                nc.vector.tensor_copy(v[:], values[c][:])
                vals[c] = v

            emit_vals_copy(0)
            emit_vals_copy(1)

            # op bank, split into 4 parts interleaved into the previous
            # chunk's write phase; ops ordered so each tree stage's inputs
            # land just before it: parts (2,0) (6,1) (7,3) (5,4).
            OPORD = (2, 0, 6, 1, 7, 3, 5, 4)
            accs, tts, ss = {}, {}, {}

            def emit_op_part(c, part):
                if part == 0:
                    accs[c] = accp.tile([128, TC], bf16, name="acc")
                    tts[c] = {}
                    ss[c] = {}
                for i in OPORD[2 * part:2 * part + 2]:
                    h = hwpsp.tile([128, TC], f32, name="hw")
                    nc.tensor.matmul(h[:], opw_t[:, i, :], vals[c][:],
                                     start=True, stop=True)
                    t = tp.tile([128, TC], bf16)
                    if i == 0:
                        nc.vector.tensor_scalar(
                            t[:], h[:], pre[0], opb_t[:, 0:1],
                            op0=alu.mult, op1=alu.add)
                    elif i == 4:
                        nc.vector.tensor_scalar(
                            t[:], h[:], pre[4], opb_t[:, 4:5],
                            op0=alu.mult, op1=alu.add)
                    else:
                        fn = [None, AF.Relu, AF.Gelu, AF.Square, None,
                              AF.Abs, AF.Tanh, AF.Tanh][i]
                        nc.scalar.activation(t[:], h[:], fn,
                                             bias=opb_t[:, i:i + 1],
                                             scale=pre[i])
                    tts[c][i] = t

            def emit_tree_stage(c, stage, fast=False):
                # mixture sum, pipelined: scaled pair-adds on DVE as inputs
                # land; closing adds on Pool so they overlap DVE drains
                # (fast=True keeps them on DVE when it is idle anyway)
                tt = tts[c]
                closing = nc.vector if fast else nc.gpsimd
                if stage == 0:
                    s0 = sp.tile([128, TC], bf16)
                    nc.vector.scalar_tensor_tensor(
                        s0[:], tt[2][:], post[2], tt[0][:],
                        op0=alu.mult, op1=alu.add)
                    ss[c][0] = s0
                elif stage == 1:
                    s1 = sp.tile([128, TC], bf16)
                    nc.vector.scalar_tensor_tensor(
                        s1[:], tt[6][:], post[6], tt[1][:],
                        op0=alu.mult, op1=alu.add)
                    u0 = sp.tile([128, TC], bf16)
                    closing.tensor_tensor(u0[:], ss[c][0][:], s1[:],
                                          op=alu.add)
                    ss[c][1] = u0
                else:
                    s2 = sp.tile([128, TC], bf16)
                    nc.vector.scalar_tensor_tensor(
                        s2[:], tt[7][:], post[7], tt[3][:],
                        op0=alu.mult, op1=alu.add)
                    s3 = sp.tile([128, TC], bf16)
                    closing.tensor_tensor(s3[:], tt[4][:], tt[5][:],
                                          op=alu.add)
                    u1 = sp.tile([128, TC], bf16)
                    closing.tensor_tensor(u1[:], s2[:], s3[:], op=alu.add)
                    closing.tensor_tensor(accs[c][:], ss[c][1][:], u1[:],
                                          op=alu.add)

            def emit_opbank(c):
                for part in range(4):
                    emit_op_part(c, part)
                    if part >= 1:
                        emit_tree_stage(c, part - 1, fast=True)
                emit_tree_stage(c, 2, fast=True)

            def emit_write(c, dr_filler, next_op):
                # write phase; next chunk's op bank and block-1 reads act as
                # PE fillers while DVE/ACT drains catch up. All stores on the
                # sync HWDGE queue (SWDGE flushes too slowly at the tail).
                for sub in range(TC // 128):
                    if next_op is not None:
                        # parts front-loaded (0+1, 2, 3, -) so the closing
                        # tree chain lands before sub3's write burst ends
                        for part in ((0, 1), (2,), (3,), ())[sub]:
                            emit_op_part(next_op, part)
                        if sub >= 1:
                            emit_tree_stage(next_op, sub - 1)
                    osb = osbp.tile([128, V], bf16)
                    for nn in range(8):
                        ops_ = hwpsp.tile([128, TC], f32, name="hw")
                        nc.tensor.matmul(ops_[:], accs[c][:, ts(sub, 128)],
                                         wwT_t[:, ts(nn, 512)],
                                         start=True, stop=True)
                        if (nn + sub) % 2 == 0:
                            nc.scalar.copy(osb[:, ts(nn, 512)], ops_[:])
                        else:
                            nc.vector.tensor_copy(osb[:, ts(nn, 512)], ops_[:])
                        row = c * (TC // 128) + sub
                        if c == 3:
                            # last chunk: store per drain on both queues so
                            # no tail backlog builds up
                            eng = nc.sync if nn % 2 == 0 else nc.gpsimd
                            eng.dma_start(
                                out.ap()[ts(row, 128), ts(nn, 512)],
                                osb[:, ts(nn, 512)])
                    if c < 3:
                        # one 8KB-line store per sub; chunk 2 avoids the slow
                        # SWDGE entirely so the bb-split barrier finds both
                        # queues drained
                        row = c * (TC // 128) + sub
                        eng = (nc.gpsimd if c < 2 and sub % 2 == 0
                               else nc.sync)
                        eng.dma_start(out.ap()[ts(row, 128), :], osb[:])
                    for k, cc in dr_filler[sub * 8:(sub + 1) * 8]:
                        emit_read(k, cc)

            emit_opbank(0)
            emit_write(0, read_mms(1), 1)
            emit_vals_copy(2)
            emit_vals_copy(3)
            emit_write(1, [], 2)
            emit_write(2, [], 3)
            emit_write(3, [], None)
    nc.compile()
    return nc


def _softmax(x, axis):
    x = np.asarray(x, np.float32)
    m = x.max(axis=axis, keepdims=True)
    e = np.exp(x - m)
    return e / e.sum(axis=axis, keepdims=True)


def kernel(x, basis, read_coeffs, write_coeffs, op_logits, op_weights,
           op_biases, out_scale):
    global LAST_RESULT
    import ml_dtypes
    from concourse.bass_utils import run_bass_kernel_spmd
    bf16 = ml_dtypes.bfloat16
    f8 = ml_dtypes.float8_e4m3

    x = np.asarray(x, np.float32)
    basis = np.asarray(basis, np.float32)
    read_coeffs = np.asarray(read_coeffs, np.float32)
    write_coeffs = np.asarray(write_coeffs, np.float32)
    op_logits = np.asarray(op_logits, np.float32)
    op_weights = np.asarray(op_weights, np.float32)
    op_biases = np.asarray(op_biases, np.float32)
    out_scale = np.float32(out_scale)

    read_w = _softmax(basis @ read_coeffs.T, axis=0)               # (V, C)
    # pack read_w to [p, vt*C]; x256 so fp8 stays in normal range, the
    # dequant 1/256 is folded into the op-bank weights below
    rw_packed = np.ascontiguousarray(
        (read_w * RW_SCALE).reshape(NV, 128, C)
        .transpose(1, 0, 2).reshape(128, NV * C))
    wwT = np.ascontiguousarray((basis @ write_coeffs.T).T) * out_scale  # (C, V)
    w = _softmax(op_logits, axis=0).astype(np.float64)

    # fold the mixture weight into scale/bias where the nonlinearity allows
    #   i: 0 ident, 1 relu, 2 gelu, 3 square, 4 neg, 5 abs, 6 tanh, 7 sigmoid
    # sigmoid(z) = 1/2 + tanh(z/2)/2: runs as Tanh with scale 0.5 and
    # post w7/2; the constant w7/2 is folded into the identity op's bias.
    pre = [w[0], w[1], 1.0, np.sqrt(w[3]), -w[4], w[5], 1.0, 0.5]
    post = [1.0, 1.0, w[2], 1.0, 1.0, 1.0, w[6], w[7] / 2.0]
    pre = [float(v) for v in pre]
    post = [float(v) for v in post]

    key = tuple(pre) + tuple(post)
    if key not in _CACHE:
        _CACHE[key] = _build(pre, post)
    nc = _CACHE[key]

    opb = (op_biases.T * np.array(pre, np.float64)[None, :]).astype(np.float32)
    # gelu/tanh biases enter before the nonlinearity unscaled; the
    # sigmoid-as-tanh op needs bias b7/2; identity carries the w7/2 constant
    opb[:, 2] = op_biases[2]
    opb[:, 6] = op_biases[6]
    opb[:, 7] = 0.5 * op_biases[7]
    opb[:, 0] += float(w[7]) / 2.0

    # opw packed to [p, i*C], with the read-path 1/RW_SCALE dequant folded in
    opw_packed = np.ascontiguousarray(
        (op_weights / RW_SCALE).transpose(1, 0, 2).reshape(C, NOPS * C))

    shared = {
        "rw": rw_packed.astype(f8),
        "wwT": wwT.astype(bf16),
        "opw": opw_packed.astype(bf16),
        "opb": np.ascontiguousarray(opb),
    }
    in_maps = []
    for b in range(B):
        m = dict(shared)
        m["xT"] = np.ascontiguousarray(x[b].T).astype(f8)
        in_maps.append(m)

    res = run_bass_kernel_spmd(nc, in_maps, core_ids=list(range(NCORES)))
    LAST_RESULT = res
    out = np.empty((B, T, V), np.float32)
    for b in range(B):
        out[b] = res.results[b]["out"].astype(np.float32)
    return out
